# revision 32
# baseline (speedup 1.0000x reference)
"""Trainium2 Bass kernel for nn_GAT_9947144257800.

2-layer GAT, B=16, N=256. Data-parallel over B across 8 NeuronCores
(2 batches per core, no collectives).

Engine strategy (from real-HW probes, not the cost model):
  - gpsimd/Pool is ~3.8us per [128,256] op on real HW (8.5x the model) --
    it is used for NOTHING.
  - All 128 z-producers per phase run on DVE in bf16 ([128,256]
    tensor_scalar ~200ns measured); ACT carries every evacuation /
    prep / softmax op (ACT+PE co-execute cleanly; 3-stream mixes tax
    the pipeline).
  - Softmax skips max-subtraction entirely (logits are bounded ~3.4;
    raw exp sums ~2.5e4, safely in f32), masking is an additive
    (adj-1)*1e30 log-mask folded into the adj transpose evacuation,
    the exp's accum_out yields masked row sums for free, and a
    [128,128] ones lhsT turns the denominator partition-reduce matmul
    into sum+broadcast in one op.
  - Leaky-relu is a single ACT Prelu (parametric_relu shares the
    exp_and_others table with Exp/Relu/Identity -- one LoadActFuncSet).
  - Cross-engine serial chains are spread over 15 breakpoints
    (>=8 producer slots ~ 1.6us apart) so each hop's input is ready
    before the in-order engine reaches it (head-of-line blocking was
    the dominant real-HW cost of the old 8-break schedule).
  - j-pairs jp and jp+32 share the accumulation step v, so one
    512-column matmul feeds both PSUM groups: 64 PE matmuls per phase
    instead of 128, halving PE instruction/Ldweights traffic (the
    residual co-execution tax tracks instruction count).

Per core / batch / layer:
  hT = Wl^T @ xT + bl                  (PE, K-tiled PSUM accumulation)
  h  = transpose(hT)                   (PE transpose)
  sid2 = [Wa|Wa]^T @ hT  (bf16)        (doubled si^T, partitions (h, j-par))
  bcols = interleave([Wb|Wb]^T @ hT + ab1)   (per-j-pair bias columns)
  z_jp = relu(sid2 + bcols[:, jp])     (DVE bf16)
  eT[2jp:2jp+2, :] = a2blockdiag^T @ z_jp    (PE bf16, stationary a2)
  lg = Prelu(eT + ab2, alpha=slope)    (ACT, from PSUM)
  lgm = lg + adjlnT                    (DVE; adjlnT = (adjT-1)*1e30)
  att, rows = Exp(lgm), accum          (ACT)
  D = ones128^T @ rows; ds = 1/D       (PE; DVE reciprocal, broadcast-free)
  x = (attT.T @ h) * ds                (PE aggregation, scaled on evacuation)
"""

import sys

if "/opt/trn_rl_repo" not in sys.path:
    sys.path.insert(0, "/opt/trn_rl_repo")

import numpy as np

B, N, IN_DIM, MEM, HID = 16, 256, 768, 300, 64
NCORES = 8
BLOC = B // NCORES  # batches per core
SLOPE = 0.01

_CACHE: dict = {}


def _build_nc(reps: int = 1):
    import concourse.mybir as mybir
    from concourse import bacc, tile

    f32 = mybir.dt.float32
    f32r = mybir.dt.float32r
    bf16 = mybir.dt.bfloat16
    AL = mybir.AluOpType
    AF = mybir.ActivationFunctionType

    nc = bacc.Bacc()

    def dp(name, shape, is_out=False):
        return nc.declare_dram_parameter(name, list(shape), f32, isOutput=is_out)

    adj_d = dp("adj", (BLOC, N, N))
    feat_d = dp("feature", (BLOC, N, IN_DIM))
    wsab0_d = dp("wsab0", (IN_DIM, 256))   # [dbl(W0@Wa) | dbl(W0@Wb)] -- gates phase 0
    w0_d = dp("w0", (IN_DIM, MEM))         # W0, first needed at the h-prep break
    wcat1_d = dp("wcat1", (384, 556))      # [W1p | dbl(W1@Wa) | dbl(W1@Wb)] (rows padded)
    bcat_d = dp("bcat", (128, 6))          # cols: bsi0 bsj0 bsi1 bsj1 ab2 -1e30
    rowcat_d = dp("rowcat", (1, 728))      # [b0row(300) | b1row(300) | ones(128)]
    a2m_d = dp("a2m", (128, 32, 64))
    id_d = dp("ident", (128, 128))
    out_d = dp("out", (BLOC, N, MEM), is_out=True)

    KT0 = [(0, 128), (1, 128), (2, 128), (3, 128), (4, 128), (5, 128)]
    KT1 = [(0, 128), (1, 128), (2, 44)]
    MC = [(0, 0, 128), (1, 128, 128), (2, 256, 44)]  # (mc, m0, cp) chunks of 300

    with tile.TileContext(nc) as tc:
        import contextlib

        with contextlib.ExitStack() as ctx:
            wp = ctx.enter_context(tc.tile_pool(name="wconst", bufs=1))
            iop = ctx.enter_context(tc.tile_pool(name="io", bufs=2))
            adjp = ctx.enter_context(tc.tile_pool(name="adjp", bufs=2))
            xtp = ctx.enter_context(tc.tile_pool(name="xtp", bufs=2))
            work = ctx.enter_context(tc.tile_pool(name="work", bufs=3))
            zpb = ctx.enter_context(tc.tile_pool(name="zpb", bufs=14))
            smp = ctx.enter_context(tc.tile_pool(name="smp", bufs=2))
            ps_eT = ctx.enter_context(tc.tile_pool(name="ps_eT", bufs=4, space="PSUM"))
            ps_mm = ctx.enter_context(tc.tile_pool(name="ps_mm", bufs=3, space="PSUM"))
            ps_sm = ctx.enter_context(tc.tile_pool(name="ps_sm", bufs=1, space="PSUM"))
            if reps > 1:
                # timing variant: run the whole kernel body `reps` times on
                # device so per-iteration time can be extracted from wall
                # clock (no NTFF profiling available under this axon setup)
                ctx.enter_context(tc.For_i(0, reps, 1))

            # ---- persistent constants + feature loads. DMA queue order =
            # emission order: identity, feature(b0) and the fused
            # s-projection weights go first (they gate the first z-phase).
            idsb = wp.tile([128, 128], f32)
            nc.sync.dma_start(idsb[:], id_d[:, :])
            fnat0 = iop.tile([128, 2, IN_DIM], f32, tag="fnat", name="fnat0")
            nc.sync.dma_start(
                fnat0[:], feat_d[0, :, :].rearrange("(i p) d -> p i d", p=128)
            )
            wsab0sb = wp.tile([128, 6, 256], f32r)
            nc.sync.dma_start(
                wsab0sb[:],
                wsab0_d[:, :].rearrange("(k p) m -> p k m", p=128).bitcast(f32r),
            )
            bcatsb = wp.tile([128, 6], f32)
            nc.sync.dma_start(bcatsb[:], bcat_d[:, :])
            a2mf = wp.tile([128, 32, 64], f32)
            nc.sync.dma_start(a2mf[:], a2m_d[:, :, :])
            fnat1 = iop.tile([128, 2, IN_DIM], f32, tag="fnat", name="fnat1")
            nc.sync.dma_start(
                fnat1[:], feat_d[1, :, :].rearrange("(i p) d -> p i d", p=128)
            )
            w0sb = wp.tile([128, 6, MEM], f32r)
            nc.sync.dma_start(
                w0sb[:],
                w0_d[:, :].rearrange("(k p) m -> p k m", p=128).bitcast(f32r),
            )
            wcat1sb = wp.tile([128, 3, 556], f32r)
            nc.sync.dma_start(
                wcat1sb[:],
                wcat1_d[:, :].rearrange("(k p) m -> p k m", p=128).bitcast(f32r),
            )
            rowcatsb = wp.tile([1, 728], f32r)
            nc.sync.dma_start(rowcatsb[:], rowcat_d[:, :].bitcast(f32r))

            # bf16 copy of the a2 block-diagonal (z matmuls are all bf16;
            # mixed 32/16-bit matmul inputs are rejected by the compiler)
            a2mb = wp.tile([128, 32, 64], bf16)
            nc.scalar.copy(a2mb[:], a2mf[:])
            ones128 = wp.tile([128, 128], f32)
            nc.vector.memset(ones128[:], 1.0)
            zcol = wp.tile([128, 1], f32)
            nc.vector.memset(zcol[:], 0.0)

            bsi = {0: bcatsb[:, 0:1], 1: bcatsb[:, 2:3]}
            bsj = {0: bcatsb[:, 1:2], 1: bcatsb[:, 3:4]}
            ab2c = bcatsb[:, 4:5]
            n1e30c = bcatsb[:, 5:6]
            brow = {0: rowcatsb[0:1, 0:MEM], 1: rowcatsb[0:1, MEM : 2 * MEM]}
            o1rr = rowcatsb[0:1, 600:728]
            wsel = {
                0: (w0sb[:, :, :], wsab0sb[:, :, 0:128], wsab0sb[:, :, 128:256]),
                1: (wcat1sb[:, :, 0:MEM], wcat1sb[:, :, 300:428], wcat1sb[:, :, 428:556]),
            }

            xT0, adjT, anats = {}, {}, {}

            def featT(b, it):
                # feature transpose, paired: 2 PE transposes share one PSUM
                # tile, one ACT evacuation moves both
                fnat = fnat0 if b == 0 else fnat1
                xTb = xT0[b]
                for kp in range(3):
                    pt = ps_mm.tile([128, MEM], f32, tag="pt")
                    for half in range(2):
                        kt = kp * 2 + half
                        nc.tensor.transpose(
                            pt[:, half * 128 : (half + 1) * 128],
                            fnat[:, it, kt * 128 : (kt + 1) * 128],
                            idsb[:],
                        )
                    dst = xTb[:, kp * 2 : kp * 2 + 2, it * 128 : (it + 1) * 128]
                    src = pt[:, 0:256].rearrange("p (k i) -> p k i", k=2)
                    nc.scalar.copy(dst, src)

            def adj_prep(b, it):
                # adjlnT[j, i-half] = (adj[i, j] - 1) * 1e30, the additive
                # log-mask, fused into the paired transpose evacuation
                if it == 0:
                    anat = iop.tile([128, 2, N], f32, tag="anat", name=f"anat{b}")
                    nc.sync.dma_start(
                        anat[:], adj_d[b, :, :].rearrange("(i p) j -> p i j", p=128)
                    )
                    adjT[b] = adjp.tile([128, 2, N], f32, tag="aT", name=f"aT{b}")
                    anats[b] = anat
                else:
                    anat = anats[b]
                aT = adjT[b]
                pt = ps_mm.tile([128, MEM], f32, tag="pt")
                for jt in range(2):
                    nc.tensor.transpose(
                        pt[:, jt * 128 : (jt + 1) * 128],
                        anat[:, it, jt * 128 : (jt + 1) * 128],
                        idsb[:],
                    )
                dst = aT[:, :, it * 128 : (it + 1) * 128]
                src = pt[:, 0:256].rearrange("p (jt i) -> p jt i", jt=2)
                nc.scalar.activation(
                    dst, src, AF.Identity, bias=n1e30c, scale=1e30,
                )

            def prep_s_sid(layer, xTb, ktiles):
                """si doubled (bf16), straight from xT via host-fused Wl@Wa."""
                wn, wsia, wsjb = wsel[layer]
                nkt = len(ktiles)
                sid2b = work.tile([128, N], bf16, tag="sid2b")
                pts = ps_mm.tile([128, MEM], f32, tag="pt")
                for kt, kr in ktiles:
                    nc.tensor.matmul(
                        pts[0:128, 0:N],
                        wsia[0:kr, kt, :],
                        xTb[0:kr, kt, :],
                        start=(kt == 0),
                        stop=(kt == nkt - 1),
                    )
                nc.scalar.activation(
                    sid2b[:], pts[0:128, 0:N], AF.Identity, bias=bsi[layer]
                )
                return sid2b

            def prep_s_bcols(layer, xTb, ktiles):
                """sj doubled -> interleaved per-j-pair bias columns (+ab1)."""
                wn, wsia, wsjb = wsel[layer]
                nkt = len(ktiles)
                bcols = work.tile([128, 128], f32, tag="bcols")
                ptj = ps_mm.tile([128, MEM], f32, tag="pt")
                for kt, kr in ktiles:
                    nc.tensor.matmul(
                        ptj[0:128, 0:N],
                        wsjb[0:kr, kt, :],
                        xTb[0:kr, kt, :],
                        start=(kt == 0),
                        stop=(kt == nkt - 1),
                    )
                lo = ptj[0:64, 0:N].rearrange("p (j two) -> p j two", two=2)
                hi = ptj[64:128, 0:N].rearrange("p (j two) -> p j two", two=2)
                nc.scalar.activation(
                    bcols[0:64, :], lo[:, :, 0], AF.Identity, bias=bsj[layer][0:64, 0:1]
                )
                nc.scalar.activation(
                    bcols[64:128, :], hi[:, :, 1], AF.Identity,
                    bias=bsj[layer][64:128, 0:1],
                )
                return bcols

            def prep_h(layer, xTb, ktiles):
                """h natural [i, m] = x @ Wl + bl; bias applied as a rank-1
                ones x b_row matmul opening each PSUM accumulation group."""
                wn, wsia, wsjb = wsel[layer]
                nkt = len(ktiles)
                h = work.tile([128, 2, MEM], f32r, tag="h")
                for it in range(2):
                    pt = ps_mm.tile([128, MEM], f32, tag="pt")
                    nc.tensor.matmul(
                        pt[0:128, 0:MEM],
                        o1rr[0:1, 0:128],
                        brow[layer],
                        start=True,
                        stop=False,
                        skip_group_check=True,
                    )
                    for kt, kr in ktiles:
                        nc.tensor.matmul(
                            pt[0:128, 0:MEM],
                            xTb[0:kr, kt, it * 128 : (it + 1) * 128],
                            wn[0:kr, kt, :],
                            start=False,
                            stop=(kt == nkt - 1),
                            skip_group_check=True,
                        )
                    nc.scalar.copy(h[:, it, :], pt[0:128, 0:MEM])
                return h

            # 15 breakpoints, >=8 slots apart: serial-chain hops get a full
            # inter-break segment of slack so in-order engines never block
            Z_BREAKS = (4, 8, 12, 16, 20, 24, 28, 32, 36, 40, 44, 48, 52, 56, 60)

            def z_phase(sid2b, bcols, fillers=(), eTs=None):
                """128 DVE bf16 producers + 64 bf16 PE reduce matmuls.
                j-pairs jp and jp+32 share the same accumulation step v, so
                their z tiles pack side by side in one [128, 512] rhs and one
                512-column matmul feeds both PSUM groups of the eT tile --
                half the PE instruction/Ldweights traffic of one-MM-per-jp.
                `fillers` are closures emitted at fixed pair breakpoints."""
                if eTs is None:
                    eTs = [
                        ps_eT.tile([64, 2, N], f32, tag="eT", name=f"eT{i}")
                        for i in range(2)
                    ]
                fills = list(fillers)
                for p in range(64):
                    for k, bp in enumerate(Z_BREAKS):
                        if p == bp and k < len(fills) and fills[k] is not None:
                            fills[k]()
                    jt, v = divmod(p, 32)
                    jp0 = 64 * jt + v
                    jp1 = jp0 + 32
                    z2 = zpb.tile([128, 2, N], bf16, tag="zb")
                    nc.vector.tensor_scalar(
                        z2[:, 0, :], sid2b[:], bcols[:, jp0 : jp0 + 1], 0.0,
                        AL.add, AL.max,
                    )
                    nc.vector.tensor_scalar(
                        z2[:, 1, :], sid2b[:], bcols[:, jp1 : jp1 + 1], 0.0,
                        AL.add, AL.max,
                    )
                    nc.tensor.matmul(
                        eTs[jt][0:64, :, :],
                        a2mb[:, v, :],
                        z2[:],
                        start=(v == 0),
                        stop=(v == 31),
                    )
                return eTs

            def prelu_tile(lg, eT, jt):
                # leaky-relu + ab2 bias fused into the PSUM evacuation
                nc.scalar.activation(
                    lg[0:64, jt, :], eT[0:64, 0, 0:N], AF.Prelu,
                    bias=ab2c[0:64, 0:1], alpha=SLOPE,
                )
                nc.scalar.activation(
                    lg[64:128, jt, :], eT[0:64, 1, 0:N], AF.Prelu,
                    bias=ab2c[64:128, 0:1], alpha=SLOPE,
                )

            def lgm_add(lg, aT, lgm):
                nc.vector.tensor_tensor(lgm[:], lg[:], aT[:], AL.add)

            def exp_den(lgm):
                # no max-subtraction: logits are bounded (~3.4 on this
                # model); raw exp sums ~2.5e4 are comfortably f32
                att = smp.tile([128, 2, N], f32r, tag="att")
                rows = smp.tile([128, 1], f32, tag="rows")
                nc.scalar.activation(
                    att[:], lgm[:], AF.Exp, accum_out=rows[:, 0:1]
                )
                return att, rows

            def den_mm(rows):
                # ones128 lhsT: partition-reduce AND broadcast in one matmul
                ptd = ps_sm.tile([128, 128], f32, tag="st")
                nc.tensor.matmul(
                    ptd[0:128, 0:1], ones128[:, :], rows[:, 0:1],
                    start=True, stop=True,
                )
                return ptd

            def recip(ptd):
                dscale = smp.tile([128, 1], f32, tag="dscale")
                nc.vector.reciprocal(dscale[:, 0:1], ptd[0:128, 0:1])
                return dscale

            def agg_l0_chunk(h, att, dscale, x1T, mc, m0, cp):
                pt = ps_mm.tile([128, MEM], f32, tag="pt")
                for jt in range(2):
                    nc.tensor.matmul(
                        pt[0:cp, 0:N],
                        h[:, jt, m0 : m0 + cp],
                        att[:, jt, :],
                        start=(jt == 0),
                        stop=(jt == 1),
                    )
                nc.scalar.activation(
                    x1T[0:cp, mc, :], pt[0:cp, 0:N], AF.Identity,
                    bias=zcol[0:cp, 0:1], scale=dscale[0:cp, 0:1],
                )

            def agg_l1_it(b, h, att, dscale, it):
                pt = ps_mm.tile([128, MEM], f32, tag="pt")
                for jt in range(2):
                    nc.tensor.matmul(
                        pt[0:128, 0:MEM],
                        att[:, jt, it * 128 : (it + 1) * 128],
                        h[:, jt, :],
                        start=(jt == 0),
                        stop=(jt == 1),
                    )
                osb = smp.tile([128, MEM], f32, tag="osb")
                nc.scalar.activation(
                    osb[:], pt[0:128, 0:MEM], AF.Identity,
                    bias=zcol[:, 0:1], scale=dscale[:, 0:1],
                )
                nc.sync.dma_start(out_d[b, it * 128 : (it + 1) * 128, :], osb[:])

            # ---- schedule
            S, Bc, H, X = {}, {}, {}, {}

            xT0[0] = xtp.tile([128, 6, N], f32r, tag="xT0", name="xT0_0")
            xT0[1] = xtp.tile([128, 6, N], f32r, tag="xT0", name="xT0_1")
            featT(0, 0)
            featT(0, 1)
            S[(0, 0)] = prep_s_sid(0, xT0[0], KT0)
            Bc[(0, 0)] = prep_s_bcols(0, xT0[0], KT0)

            eT00 = z_phase(
                S[(0, 0)],
                Bc[(0, 0)],
                fillers=(
                    lambda: featT(1, 0),
                    lambda: featT(1, 1),
                    lambda: S.__setitem__((1, 0), prep_s_sid(0, xT0[1], KT0)),
                    lambda: Bc.__setitem__((1, 0), prep_s_bcols(0, xT0[1], KT0)),
                    lambda: adj_prep(0, 0),
                    lambda: adj_prep(0, 1),
                    lambda: adj_prep(1, 0),
                    lambda: adj_prep(1, 1),
                    lambda: H.__setitem__((0, 0), prep_h(0, xT0[0], KT0)),
                    lambda: H.__setitem__((1, 0), prep_h(0, xT0[1], KT0)),
                ),
            )

            def mk_fillers(eTs, b, layer, nxt):
                """softmax chain spread one hop per breakpoint, then agg
                chunks, then next-(batch,layer) preps."""
                box = {}
                lg = smp.tile([128, 2, N], f32, tag="lg", name=f"lg{b}{layer}")
                lgm = smp.tile([128, 2, N], f32, tag="lgm", name=f"lgm{b}{layer}")

                fl = [
                    lambda: prelu_tile(lg, eTs[0], 0),
                    lambda: prelu_tile(lg, eTs[1], 1),
                    lambda: lgm_add(lg, adjT[b], lgm),
                    lambda: box.update(zip(("att", "rows"), exp_den(lgm))),
                    lambda: box.update(ptd=den_mm(box["rows"])),
                    lambda: box.update(ds=recip(box["ptd"])),
                ]
                if layer == 0:
                    x1T = xtp.tile([128, 3, N], f32r, tag="x1T", name=f"x1T{b}")
                    X[b] = x1T
                    for mc, m0, cp in MC:
                        fl.append(
                            lambda mc=mc, m0=m0, cp=cp: agg_l0_chunk(
                                H[(b, 0)], box["att"], box["ds"], x1T, mc, m0, cp
                            )
                        )
                else:
                    fl.append(lambda: agg_l1_it(b, H[(b, 1)], box["att"], box["ds"], 0))
                    fl.append(lambda: agg_l1_it(b, H[(b, 1)], box["att"], box["ds"], 1))
                    fl.append(None)
                if nxt:
                    fl.append(lambda: S.__setitem__((b, 1), prep_s_sid(1, X[b], KT1)))
                    fl.append(lambda: Bc.__setitem__((b, 1), prep_s_bcols(1, X[b], KT1)))
                    fl.append(lambda: H.__setitem__((b, 1), prep_h(1, X[b], KT1)))
                return fl, box

            fl1, _ = mk_fillers(eT00, 0, 0, True)
            eT10 = z_phase(S[(1, 0)], Bc[(1, 0)], fillers=fl1)
            fl2, _ = mk_fillers(eT10, 1, 0, True)
            eT01 = z_phase(S[(0, 1)], Bc[(0, 1)], fillers=fl2)
            # last phase: own eT[0] completes after jp=63, so its prelu can
            # start inside the phase
            eT11 = [
                ps_eT.tile([64, 2, N], f32, tag="eT", name=f"eT11_{i}")
                for i in range(2)
            ]
            lg11 = smp.tile([128, 2, N], f32, tag="lg", name="lg11")
            lgm11 = smp.tile([128, 2, N], f32, tag="lgm", name="lgm11")
            fl3, _ = mk_fillers(eT01, 0, 1, False)
            fl3 = fl3 + [None] * (13 - len(fl3))
            fl3[11] = lambda: prelu_tile(lg11, eT11[0], 0)
            z_phase(S[(1, 1)], Bc[(1, 1)], fillers=fl3, eTs=eT11)
            prelu_tile(lg11, eT11[1], 1)
            lgm_add(lg11, adjT[1], lgm11)
            att11, rows11 = exp_den(lgm11)
            ptd11 = den_mm(rows11)
            ds11 = recip(ptd11)
            agg_l1_it(1, H[(1, 1)], att11, ds11, 0)
            agg_l1_it(1, H[(1, 1)], att11, ds11, 1)

    nc.compile()
    return nc


def _host_params(W0, b0, W1, b1, A1, ab1, A2, ab2):
    f = np.float32
    d = np.float64
    Wa, Wb = np.asarray(A1[:MEM], d), np.asarray(A1[MEM:], d)
    a2 = np.asarray(A2, d)[:, 0]
    W0 = np.asarray(W0, d)
    W1 = np.asarray(W1, d)
    b0 = np.asarray(b0, d)
    b1 = np.asarray(b1, d)
    ab1 = np.asarray(ab1, d)

    def pad_rows(x, rows):
        out = np.zeros((rows,) + x.shape[1:], f)
        out[: x.shape[0]] = x
        return out

    def dbl(x):  # [K, 64] -> [K, 128] doubled columns
        return np.concatenate([x, x], axis=1)

    def dupcol(v):  # [64] -> [128] doubled
        return np.concatenate([v, v]).astype(f)

    ab2v = float(np.asarray(ab2, f).reshape(-1)[0])
    a2m = np.zeros((128, 32, 64), f)
    for v in range(32):
        a2m[0:64, v, 2 * v] = a2
        a2m[64:128, v, 2 * v + 1] = a2

    wsab0 = np.concatenate(
        [dbl(W0 @ Wa).astype(f), dbl(W0 @ Wb).astype(f)], axis=1
    )
    wcat1 = np.concatenate(
        [
            pad_rows(W1.astype(f), 384),
            pad_rows(dbl(W1 @ Wa).astype(f), 384),
            pad_rows(dbl(W1 @ Wb).astype(f), 384),
        ],
        axis=1,
    )
    bcat = np.stack(
        [
            dupcol(b0 @ Wa),
            dupcol(b0 @ Wb + ab1),
            dupcol(b1 @ Wa),
            dupcol(b1 @ Wb + ab1),
            np.full(128, ab2v, f),
            np.full(128, -1e30, f),
        ],
        axis=1,
    )
    rowcat = np.concatenate(
        [b0.astype(f), b1.astype(f), np.ones(128, f)]
    )[None, :]
    return dict(
        wsab0=np.ascontiguousarray(wsab0, f),
        w0=np.ascontiguousarray(W0.astype(f), f),
        wcat1=np.ascontiguousarray(wcat1, f),
        bcat=np.ascontiguousarray(bcat, f),
        rowcat=np.ascontiguousarray(rowcat, f),
        a2m=a2m,
        ident=np.eye(128, dtype=f),
    )


def get_nc(reps: int = 1):
    key = f"nc{reps}"
    if key not in _CACHE:
        _CACHE[key] = _build_nc(reps)
    return _CACHE[key]


def kernel(adj, feature, W0, b0, W1, b1, A1, ab1, A2, ab2):
    from concourse.bass_utils import run_bass_kernel_spmd

    nc = get_nc()
    params = _host_params(W0, b0, W1, b1, A1, ab1, A2, ab2)
    f = np.float32
    adj = np.ascontiguousarray(adj, f)
    feature = np.ascontiguousarray(feature, f)
    in_maps = []
    for c in range(NCORES):
        m = dict(params)
        m["adj"] = adj[c * BLOC : (c + 1) * BLOC]
        m["feature"] = feature[c * BLOC : (c + 1) * BLOC]
        in_maps.append(m)
    try:
        r = run_bass_kernel_spmd(nc, in_maps, list(range(NCORES)))
    except Exception:
        # a freshly-reprogrammed device occasionally reports
        # NRT_EXEC_UNIT_UNRECOVERABLE once and recovers on retry
        r = run_bass_kernel_spmd(nc, in_maps, list(range(NCORES)))
    out = np.concatenate([r.results[c]["out"] for c in range(NCORES)], axis=0)
    return out.astype(np.float32)


# revision 34
# speedup vs baseline: 1.0751x; 1.0751x over previous
"""Trainium2 Bass kernel for nn_GAT_9947144257800.

2-layer GAT, B=16, N=256. Data-parallel over B across 8 NeuronCores
(2 batches per core, no collectives).

Engine strategy (from real-HW probes, not the cost model):
  - gpsimd/Pool is ~3.8us per [128,256] op on real HW (8.5x the model) --
    it is used for NOTHING.
  - All 128 z-producers per phase run on DVE in bf16 ([128,256]
    tensor_scalar ~200ns measured); ACT carries every evacuation /
    prep / softmax op (ACT+PE co-execute cleanly; 3-stream mixes tax
    the pipeline).
  - Softmax skips max-subtraction entirely (logits are bounded ~3.4;
    raw exp sums ~2.5e4, safely in f32), masking is an additive
    (adj-1)*1e30 log-mask folded into the adj transpose evacuation,
    the exp's accum_out yields masked row sums for free, and a
    [128,128] ones lhsT turns the denominator partition-reduce matmul
    into sum+broadcast in one op.
  - Leaky-relu is a single ACT Prelu (parametric_relu shares the
    exp_and_others table with Exp/Relu/Identity -- one LoadActFuncSet).
  - Cross-engine serial chains are spread over 15 breakpoints
    (>=8 producer slots ~ 1.6us apart) so each hop's input is ready
    before the in-order engine reaches it (head-of-line blocking was
    the dominant real-HW cost of the old 8-break schedule).
  - j-pairs jp and jp+32 share the accumulation step v, so one
    512-column matmul feeds both PSUM groups: 64 PE matmuls per phase
    instead of 128, halving PE instruction/Ldweights traffic (the
    residual co-execution tax tracks instruction count).

Per core / batch / layer:
  hT = Wl^T @ xT + bl                  (PE, K-tiled PSUM accumulation)
  h  = transpose(hT)                   (PE transpose)
  sid2 = [Wa|Wa]^T @ hT  (bf16)        (doubled si^T, partitions (h, j-par))
  bcols = interleave([Wb|Wb]^T @ hT + ab1)   (per-j-pair bias columns)
  z_jp = relu(sid2 + bcols[:, jp])     (DVE bf16)
  eT[2jp:2jp+2, :] = a2blockdiag^T @ z_jp    (PE bf16, stationary a2)
  lg = Prelu(eT + ab2, alpha=slope)    (ACT, from PSUM)
  lgm = lg + adjlnT                    (DVE; adjlnT = (adjT-1)*1e30)
  att, rows = Exp(lgm), accum          (ACT)
  D = ones128^T @ rows; ds = 1/D       (PE; DVE reciprocal, broadcast-free)
  x = (attT.T @ h) * ds                (PE aggregation, scaled on evacuation)
"""

import sys

if "/opt/trn_rl_repo" not in sys.path:
    sys.path.insert(0, "/opt/trn_rl_repo")

import numpy as np

B, N, IN_DIM, MEM, HID = 16, 256, 768, 300, 64
NCORES = 8
BLOC = B // NCORES  # batches per core
SLOPE = 0.01

_CACHE: dict = {}


def _build_nc(reps: int = 1):
    import concourse.mybir as mybir
    from concourse import bacc, tile

    f32 = mybir.dt.float32
    f32r = mybir.dt.float32r
    bf16 = mybir.dt.bfloat16
    AL = mybir.AluOpType
    AF = mybir.ActivationFunctionType

    nc = bacc.Bacc()

    def dp(name, shape, is_out=False):
        return nc.declare_dram_parameter(name, list(shape), f32, isOutput=is_out)

    adj_d = dp("adj", (BLOC, N, N))
    feat_d = dp("feature", (BLOC, N, IN_DIM))
    wsab0_d = dp("wsab0", (IN_DIM, 256))   # [dbl(W0@Wa) | dbl(W0@Wb)] -- gates phase 0
    w0_d = dp("w0", (IN_DIM, MEM))         # W0, first needed at the h-prep break
    wcat1_d = dp("wcat1", (384, 556))      # [W1p | dbl(W1@Wa) | dbl(W1@Wb)] (rows padded)
    bcat_d = dp("bcat", (128, 6))          # cols: bsi0 bsj0 bsi1 bsj1 ab2 -1e30
    rowcat_d = dp("rowcat", (1, 728))      # [b0row(300) | b1row(300) | ones(128)]
    a2m_d = dp("a2m", (128, 32, 64))
    id_d = dp("ident", (128, 128))
    out_d = dp("out", (BLOC, N, MEM), is_out=True)

    KT0 = [(0, 128), (1, 128), (2, 128), (3, 128), (4, 128), (5, 128)]
    KT1 = [(0, 128), (1, 128), (2, 44)]
    MC = [(0, 0, 128), (1, 128, 128), (2, 256, 44)]  # (mc, m0, cp) chunks of 300

    with tile.TileContext(nc) as tc:
        import contextlib

        with contextlib.ExitStack() as ctx:
            wp = ctx.enter_context(tc.tile_pool(name="wconst", bufs=1))
            iop = ctx.enter_context(tc.tile_pool(name="io", bufs=2))
            adjp = ctx.enter_context(tc.tile_pool(name="adjp", bufs=2))
            xtp = ctx.enter_context(tc.tile_pool(name="xtp", bufs=2))
            work = ctx.enter_context(tc.tile_pool(name="work", bufs=3))
            zpb = ctx.enter_context(tc.tile_pool(name="zpb", bufs=14))
            smp = ctx.enter_context(tc.tile_pool(name="smp", bufs=2))
            ps_eT = ctx.enter_context(tc.tile_pool(name="ps_eT", bufs=4, space="PSUM"))
            ps_mm = ctx.enter_context(tc.tile_pool(name="ps_mm", bufs=3, space="PSUM"))
            ps_sm = ctx.enter_context(tc.tile_pool(name="ps_sm", bufs=1, space="PSUM"))
            if reps > 1:
                # timing variant: run the whole kernel body `reps` times on
                # device so per-iteration time can be extracted from wall
                # clock (no NTFF profiling available under this axon setup)
                ctx.enter_context(tc.For_i(0, reps, 1))

            # ---- persistent constants + feature loads. DMA queue order =
            # emission order: identity, feature(b0) and the fused
            # s-projection weights go first (they gate the first z-phase).
            idsb = wp.tile([128, 128], f32)
            nc.sync.dma_start(idsb[:], id_d[:, :])
            fnat0 = iop.tile([128, 2, IN_DIM], f32, tag="fnat", name="fnat0")
            nc.sync.dma_start(
                fnat0[:], feat_d[0, :, :].rearrange("(i p) d -> p i d", p=128)
            )
            wsab0sb = wp.tile([128, 6, 256], f32r)
            nc.sync.dma_start(
                wsab0sb[:],
                wsab0_d[:, :].rearrange("(k p) m -> p k m", p=128).bitcast(f32r),
            )
            bcatsb = wp.tile([128, 6], f32)
            nc.sync.dma_start(bcatsb[:], bcat_d[:, :])
            a2mf = wp.tile([128, 32, 64], f32)
            nc.sync.dma_start(a2mf[:], a2m_d[:, :, :])
            fnat1 = iop.tile([128, 2, IN_DIM], f32, tag="fnat", name="fnat1")
            nc.sync.dma_start(
                fnat1[:], feat_d[1, :, :].rearrange("(i p) d -> p i d", p=128)
            )
            w0sb = wp.tile([128, 6, MEM], f32r)
            nc.sync.dma_start(
                w0sb[:],
                w0_d[:, :].rearrange("(k p) m -> p k m", p=128).bitcast(f32r),
            )
            wcat1sb = wp.tile([128, 3, 556], f32r)
            nc.sync.dma_start(
                wcat1sb[:],
                wcat1_d[:, :].rearrange("(k p) m -> p k m", p=128).bitcast(f32r),
            )
            rowcatsb = wp.tile([1, 728], f32r)
            nc.sync.dma_start(rowcatsb[:], rowcat_d[:, :].bitcast(f32r))

            # bf16 copy of the a2 block-diagonal (z matmuls are all bf16;
            # mixed 32/16-bit matmul inputs are rejected by the compiler)
            a2mb = wp.tile([128, 32, 64], bf16)
            nc.scalar.copy(a2mb[:], a2mf[:])
            ones128 = wp.tile([128, 128], f32)
            nc.vector.memset(ones128[:], 1.0)
            zcol = wp.tile([128, 1], f32)
            nc.vector.memset(zcol[:], 0.0)

            bsi = {0: bcatsb[:, 0:1], 1: bcatsb[:, 2:3]}
            bsj = {0: bcatsb[:, 1:2], 1: bcatsb[:, 3:4]}
            ab2c = bcatsb[:, 4:5]
            n1e30c = bcatsb[:, 5:6]
            brow = {0: rowcatsb[0:1, 0:MEM], 1: rowcatsb[0:1, MEM : 2 * MEM]}
            o1rr = rowcatsb[0:1, 600:728]
            wsel = {
                0: (w0sb[:, :, :], wsab0sb[:, :, 0:128], wsab0sb[:, :, 128:256]),
                1: (wcat1sb[:, :, 0:MEM], wcat1sb[:, :, 300:428], wcat1sb[:, :, 428:556]),
            }

            xT0, adjT, anats = {}, {}, {}

            def featT(b, it):
                # feature transpose, paired: 2 PE transposes share one PSUM
                # tile, one ACT evacuation moves both
                fnat = fnat0 if b == 0 else fnat1
                xTb = xT0[b]
                for kp in range(3):
                    pt = ps_mm.tile([128, MEM], f32, tag="pt")
                    for half in range(2):
                        kt = kp * 2 + half
                        nc.tensor.transpose(
                            pt[:, half * 128 : (half + 1) * 128],
                            fnat[:, it, kt * 128 : (kt + 1) * 128],
                            idsb[:],
                        )
                    dst = xTb[:, kp * 2 : kp * 2 + 2, it * 128 : (it + 1) * 128]
                    src = pt[:, 0:256].rearrange("p (k i) -> p k i", k=2)
                    nc.scalar.copy(dst, src)

            def adj_prep(b, it):
                # adjlnT[j, i-half] = (adj[i, j] - 1) * 1e30, the additive
                # log-mask, fused into the paired transpose evacuation
                if it == 0:
                    anat = iop.tile([128, 2, N], f32, tag="anat", name=f"anat{b}")
                    nc.sync.dma_start(
                        anat[:], adj_d[b, :, :].rearrange("(i p) j -> p i j", p=128)
                    )
                    adjT[b] = adjp.tile([128, 2, N], f32, tag="aT", name=f"aT{b}")
                    anats[b] = anat
                else:
                    anat = anats[b]
                aT = adjT[b]
                pt = ps_mm.tile([128, MEM], f32, tag="pt")
                for jt in range(2):
                    nc.tensor.transpose(
                        pt[:, jt * 128 : (jt + 1) * 128],
                        anat[:, it, jt * 128 : (jt + 1) * 128],
                        idsb[:],
                    )
                dst = aT[:, :, it * 128 : (it + 1) * 128]
                src = pt[:, 0:256].rearrange("p (jt i) -> p jt i", jt=2)
                nc.scalar.activation(
                    dst, src, AF.Identity, bias=n1e30c, scale=1e30,
                )

            def prep_s_sid(layer, xTb, ktiles):
                """si doubled (bf16), straight from xT via host-fused Wl@Wa."""
                wn, wsia, wsjb = wsel[layer]
                nkt = len(ktiles)
                sid2b = work.tile([128, N], bf16, tag="sid2b")
                pts = ps_mm.tile([128, MEM], f32, tag="pt")
                for kt, kr in ktiles:
                    nc.tensor.matmul(
                        pts[0:128, 0:N],
                        wsia[0:kr, kt, :],
                        xTb[0:kr, kt, :],
                        start=(kt == 0),
                        stop=(kt == nkt - 1),
                    )
                nc.scalar.activation(
                    sid2b[:], pts[0:128, 0:N], AF.Identity, bias=bsi[layer]
                )
                return sid2b

            def prep_s_bcols(layer, xTb, ktiles):
                """sj doubled -> interleaved per-j-pair bias columns (+ab1)."""
                wn, wsia, wsjb = wsel[layer]
                nkt = len(ktiles)
                bcols = work.tile([128, 128], f32, tag="bcols")
                ptj = ps_mm.tile([128, MEM], f32, tag="pt")
                for kt, kr in ktiles:
                    nc.tensor.matmul(
                        ptj[0:128, 0:N],
                        wsjb[0:kr, kt, :],
                        xTb[0:kr, kt, :],
                        start=(kt == 0),
                        stop=(kt == nkt - 1),
                    )
                lo = ptj[0:64, 0:N].rearrange("p (j two) -> p j two", two=2)
                hi = ptj[64:128, 0:N].rearrange("p (j two) -> p j two", two=2)
                nc.scalar.activation(
                    bcols[0:64, :], lo[:, :, 0], AF.Identity, bias=bsj[layer][0:64, 0:1]
                )
                nc.scalar.activation(
                    bcols[64:128, :], hi[:, :, 1], AF.Identity,
                    bias=bsj[layer][64:128, 0:1],
                )
                return bcols

            def prep_h(layer, xTb, ktiles):
                """h natural [i, m] = x @ Wl + bl; bias applied as a rank-1
                ones x b_row matmul opening each PSUM accumulation group."""
                wn, wsia, wsjb = wsel[layer]
                nkt = len(ktiles)
                h = work.tile([128, 2, MEM], f32r, tag="h")
                for it in range(2):
                    pt = ps_mm.tile([128, MEM], f32, tag="pt")
                    nc.tensor.matmul(
                        pt[0:128, 0:MEM],
                        o1rr[0:1, 0:128],
                        brow[layer],
                        start=True,
                        stop=False,
                        skip_group_check=True,
                    )
                    for kt, kr in ktiles:
                        nc.tensor.matmul(
                            pt[0:128, 0:MEM],
                            xTb[0:kr, kt, it * 128 : (it + 1) * 128],
                            wn[0:kr, kt, :],
                            start=False,
                            stop=(kt == nkt - 1),
                            skip_group_check=True,
                        )
                    nc.scalar.copy(h[:, it, :], pt[0:128, 0:MEM])
                return h

            # 15 breakpoints, >=8 slots apart: serial-chain hops get a full
            # inter-break segment of slack so in-order engines never block
            Z_BREAKS = (4, 8, 12, 16, 20, 24, 28, 32, 36, 40, 44, 48, 52, 56, 60)

            def z_phase(sid2b, bcols, fillers=(), eTs=None):
                """128 DVE bf16 producers + 64 bf16 PE reduce matmuls.
                j-pairs jp and jp+32 share the same accumulation step v, so
                their z tiles pack side by side in one [128, 512] rhs and one
                512-column matmul feeds both PSUM groups of the eT tile --
                half the PE instruction/Ldweights traffic of one-MM-per-jp.
                `fillers` are closures emitted at fixed pair breakpoints."""
                if eTs is None:
                    eTs = [
                        ps_eT.tile([64, 2, N], f32, tag="eT", name=f"eT{i}")
                        for i in range(2)
                    ]
                fills = list(fillers)
                for p in range(64):
                    for k, bp in enumerate(Z_BREAKS):
                        if p == bp and k < len(fills) and fills[k] is not None:
                            fills[k]()
                    jt, v = divmod(p, 32)
                    jp0 = 64 * jt + v
                    jp1 = jp0 + 32
                    z2 = zpb.tile([128, 2, N], bf16, tag="zb")
                    nc.vector.tensor_scalar(
                        z2[:, 0, :], sid2b[:], bcols[:, jp0 : jp0 + 1], 0.0,
                        AL.add, AL.max,
                    )
                    nc.vector.tensor_scalar(
                        z2[:, 1, :], sid2b[:], bcols[:, jp1 : jp1 + 1], 0.0,
                        AL.add, AL.max,
                    )
                    nc.tensor.matmul(
                        eTs[jt][0:64, :, :],
                        a2mb[:, v, :],
                        z2[:],
                        start=(v == 0),
                        stop=(v == 31),
                    )
                return eTs

            def prelu_tile(lg, eT, jt):
                # leaky-relu + ab2 bias fused into the PSUM evacuation
                nc.scalar.activation(
                    lg[0:64, jt, :], eT[0:64, 0, 0:N], AF.Prelu,
                    bias=ab2c[0:64, 0:1], alpha=SLOPE,
                )
                nc.scalar.activation(
                    lg[64:128, jt, :], eT[0:64, 1, 0:N], AF.Prelu,
                    bias=ab2c[64:128, 0:1], alpha=SLOPE,
                )

            def lgm_add(lg, aT, lgm):
                nc.vector.tensor_tensor(lgm[:], lg[:], aT[:], AL.add)

            def exp_den(lgm):
                # no max-subtraction: logits are bounded (~3.4 on this
                # model); raw exp sums ~2.5e4 are comfortably f32
                att = smp.tile([128, 2, N], f32r, tag="att")
                rows = smp.tile([128, 1], f32, tag="rows")
                nc.scalar.activation(
                    att[:], lgm[:], AF.Exp, accum_out=rows[:, 0:1]
                )
                return att, rows

            def den_mm(rows):
                # ones128 lhsT: partition-reduce AND broadcast in one matmul
                ptd = ps_sm.tile([128, 128], f32, tag="st")
                nc.tensor.matmul(
                    ptd[0:128, 0:1], ones128[:, :], rows[:, 0:1],
                    start=True, stop=True,
                )
                return ptd

            def recip(ptd):
                dscale = smp.tile([128, 1], f32, tag="dscale")
                nc.vector.reciprocal(dscale[:, 0:1], ptd[0:128, 0:1])
                return dscale

            def agg_l0_chunk(h, att, dscale, x1T, mc, m0, cp):
                pt = ps_mm.tile([128, MEM], f32, tag="pt")
                for jt in range(2):
                    nc.tensor.matmul(
                        pt[0:cp, 0:N],
                        h[:, jt, m0 : m0 + cp],
                        att[:, jt, :],
                        start=(jt == 0),
                        stop=(jt == 1),
                    )
                nc.scalar.activation(
                    x1T[0:cp, mc, :], pt[0:cp, 0:N], AF.Identity,
                    bias=zcol[0:cp, 0:1], scale=dscale[0:cp, 0:1],
                )

            def agg_l1_it(b, h, att, dscale, it):
                pt = ps_mm.tile([128, MEM], f32, tag="pt")
                for jt in range(2):
                    nc.tensor.matmul(
                        pt[0:128, 0:MEM],
                        att[:, jt, it * 128 : (it + 1) * 128],
                        h[:, jt, :],
                        start=(jt == 0),
                        stop=(jt == 1),
                    )
                osb = smp.tile([128, MEM], f32, tag="osb")
                nc.scalar.activation(
                    osb[:], pt[0:128, 0:MEM], AF.Identity,
                    bias=zcol[:, 0:1], scale=dscale[:, 0:1],
                )
                nc.sync.dma_start(out_d[b, it * 128 : (it + 1) * 128, :], osb[:])

            # ---- schedule
            S, Bc, H, X = {}, {}, {}, {}

            xT0[0] = xtp.tile([128, 6, N], f32r, tag="xT0", name="xT0_0")
            xT0[1] = xtp.tile([128, 6, N], f32r, tag="xT0", name="xT0_1")
            featT(0, 0)
            featT(0, 1)
            S[(0, 0)] = prep_s_sid(0, xT0[0], KT0)
            Bc[(0, 0)] = prep_s_bcols(0, xT0[0], KT0)

            eT00 = z_phase(
                S[(0, 0)],
                Bc[(0, 0)],
                fillers=(
                    lambda: featT(1, 0),
                    lambda: featT(1, 1),
                    lambda: S.__setitem__((1, 0), prep_s_sid(0, xT0[1], KT0)),
                    lambda: Bc.__setitem__((1, 0), prep_s_bcols(0, xT0[1], KT0)),
                    lambda: adj_prep(0, 0),
                    lambda: adj_prep(0, 1),
                    lambda: adj_prep(1, 0),
                    lambda: adj_prep(1, 1),
                    lambda: H.__setitem__((0, 0), prep_h(0, xT0[0], KT0)),
                    lambda: H.__setitem__((1, 0), prep_h(0, xT0[1], KT0)),
                ),
            )

            def mk_fillers(eTs, b, layer, nxt):
                """softmax chain spread one hop per breakpoint, then agg
                chunks, then next-(batch,layer) preps."""
                box = {}
                lg = smp.tile([128, 2, N], f32, tag="lg", name=f"lg{b}{layer}")
                lgm = smp.tile([128, 2, N], f32, tag="lgm", name=f"lgm{b}{layer}")

                fl = [
                    lambda: prelu_tile(lg, eTs[0], 0),
                    lambda: prelu_tile(lg, eTs[1], 1),
                    lambda: lgm_add(lg, adjT[b], lgm),
                    lambda: box.update(zip(("att", "rows"), exp_den(lgm))),
                    lambda: box.update(ptd=den_mm(box["rows"])),
                    lambda: box.update(ds=recip(box["ptd"])),
                ]
                if layer == 0:
                    x1T = xtp.tile([128, 3, N], f32r, tag="x1T", name=f"x1T{b}")
                    X[b] = x1T
                    for mc, m0, cp in MC:
                        fl.append(
                            lambda mc=mc, m0=m0, cp=cp: agg_l0_chunk(
                                H[(b, 0)], box["att"], box["ds"], x1T, mc, m0, cp
                            )
                        )
                else:
                    fl.append(lambda: agg_l1_it(b, H[(b, 1)], box["att"], box["ds"], 0))
                    fl.append(lambda: agg_l1_it(b, H[(b, 1)], box["att"], box["ds"], 1))
                    fl.append(None)
                if nxt:
                    fl.append(lambda: S.__setitem__((b, 1), prep_s_sid(1, X[b], KT1)))
                    fl.append(lambda: Bc.__setitem__((b, 1), prep_s_bcols(1, X[b], KT1)))
                    fl.append(lambda: H.__setitem__((b, 1), prep_h(1, X[b], KT1)))
                return fl, box

            fl1, _ = mk_fillers(eT00, 0, 0, True)
            eT10 = z_phase(S[(1, 0)], Bc[(1, 0)], fillers=fl1)
            fl2, _ = mk_fillers(eT10, 1, 0, True)
            eT01 = z_phase(S[(0, 1)], Bc[(0, 1)], fillers=fl2)
            # last phase: own eT[0] completes after jp=63, so its prelu can
            # start inside the phase
            eT11 = [
                ps_eT.tile([64, 2, N], f32, tag="eT", name=f"eT11_{i}")
                for i in range(2)
            ]
            lg11 = smp.tile([128, 2, N], f32, tag="lg", name="lg11")
            lgm11 = smp.tile([128, 2, N], f32, tag="lgm", name="lgm11")
            fl3, _ = mk_fillers(eT01, 0, 1, False)
            fl3 = fl3 + [None] * (13 - len(fl3))
            fl3[11] = lambda: prelu_tile(lg11, eT11[0], 0)
            z_phase(S[(1, 1)], Bc[(1, 1)], fillers=fl3, eTs=eT11)
            prelu_tile(lg11, eT11[1], 1)
            lgm_add(lg11, adjT[1], lgm11)
            att11, rows11 = exp_den(lgm11)
            ptd11 = den_mm(rows11)
            ds11 = recip(ptd11)
            agg_l1_it(1, H[(1, 1)], att11, ds11, 0)
            agg_l1_it(1, H[(1, 1)], att11, ds11, 1)

    nc.compile()
    return nc


def _host_params(W0, b0, W1, b1, A1, ab1, A2, ab2):
    f = np.float32
    d = np.float64
    Wa, Wb = np.asarray(A1[:MEM], d), np.asarray(A1[MEM:], d)
    a2 = np.asarray(A2, d)[:, 0]
    W0 = np.asarray(W0, d)
    W1 = np.asarray(W1, d)
    b0 = np.asarray(b0, d)
    b1 = np.asarray(b1, d)
    ab1 = np.asarray(ab1, d)

    def pad_rows(x, rows):
        out = np.zeros((rows,) + x.shape[1:], f)
        out[: x.shape[0]] = x
        return out

    def dbl(x):  # [K, 64] -> [K, 128] doubled columns
        return np.concatenate([x, x], axis=1)

    def dupcol(v):  # [64] -> [128] doubled
        return np.concatenate([v, v]).astype(f)

    ab2v = float(np.asarray(ab2, f).reshape(-1)[0])
    a2m = np.zeros((128, 32, 64), f)
    for v in range(32):
        a2m[0:64, v, 2 * v] = a2
        a2m[64:128, v, 2 * v + 1] = a2

    wsab0 = np.concatenate(
        [dbl(W0 @ Wa).astype(f), dbl(W0 @ Wb).astype(f)], axis=1
    )
    wcat1 = np.concatenate(
        [
            pad_rows(W1.astype(f), 384),
            pad_rows(dbl(W1 @ Wa).astype(f), 384),
            pad_rows(dbl(W1 @ Wb).astype(f), 384),
        ],
        axis=1,
    )
    bcat = np.stack(
        [
            dupcol(b0 @ Wa),
            dupcol(b0 @ Wb + ab1),
            dupcol(b1 @ Wa),
            dupcol(b1 @ Wb + ab1),
            np.full(128, ab2v, f),
            np.full(128, -1e30, f),
        ],
        axis=1,
    )
    rowcat = np.concatenate(
        [b0.astype(f), b1.astype(f), np.ones(128, f)]
    )[None, :]
    return dict(
        wsab0=np.ascontiguousarray(wsab0, f),
        w0=np.ascontiguousarray(W0.astype(f), f),
        wcat1=np.ascontiguousarray(wcat1, f),
        bcat=np.ascontiguousarray(bcat, f),
        rowcat=np.ascontiguousarray(rowcat, f),
        a2m=a2m,
        ident=np.eye(128, dtype=f),
    )


def get_nc(reps: int = 1):
    key = f"nc{reps}"
    if key not in _CACHE:
        _CACHE[key] = _build_nc(reps)
    return _CACHE[key]


def kernel(adj, feature, W0, b0, W1, b1, A1, ab1, A2, ab2):
    from concourse.bass_utils import run_bass_kernel_spmd

    nc = get_nc()
    params = _host_params(W0, b0, W1, b1, A1, ab1, A2, ab2)
    f = np.float32
    adj = np.ascontiguousarray(adj, f)
    feature = np.ascontiguousarray(feature, f)
    in_maps = []
    for c in range(NCORES):
        m = dict(params)
        m["adj"] = adj[c * BLOC : (c + 1) * BLOC]
        m["feature"] = feature[c * BLOC : (c + 1) * BLOC]
        in_maps.append(m)
    try:
        r = run_bass_kernel_spmd(nc, in_maps, list(range(NCORES)))
    except Exception:
        # a freshly-reprogrammed device occasionally reports
        # NRT_EXEC_UNIT_UNRECOVERABLE once and recovers on retry
        r = run_bass_kernel_spmd(nc, in_maps, list(range(NCORES)))
    out = np.concatenate([r.results[c]["out"] for c in range(NCORES)], axis=0)
    return out.astype(np.float32)
